# revision 1
# baseline (speedup 1.0000x reference)
import numpy as np

E, F, H = 8, 4096, 2048
B, S, K = 2, 1024, 4
T = B * S

FB = F // 128       # 32 f-tiles
HB = H // 128       # 16 h-tiles
ICW = 512           # output column chunk
IC = H // ICW       # 4 output col chunks
TSH = T // E        # 256 rows per core after ReduceScatter

_STATE = {}


def _chunks(cap):
    out, c0 = [], 0
    while c0 < cap:
        cw = min(512, cap - c0)
        out.append((c0, cw))
        c0 += cw
    return out


def _build_nc(cap, reps=1, ncols=None, combine=True, phase1=True, phase2=True,
              pair=True, dedup=True, yshare=True, wbufs=2, ztrim=False):
    # ncols: real (unpadded) token columns; cols [ncols, cap) of h are never
    # computed — their phase-2 outputs land in the trash row via the scatter
    # index padding, so garbage there is harmless.
    #
    # Combine path: each rep scatters its expert's token rows into a
    # double-buffered [T+128, H] DRAM staging buffer, then one ReduceScatter
    # sums across the 8 cores and writes each core's 256-row shard directly
    # into the ExternalOutput. Double buffering (dram pool bufs=2) lets rep
    # i's ReduceScatter run concurrently with rep i+1's compute, so in
    # steady state the collective is fully hidden.
    import concourse.bacc as bacc
    import concourse.bass as bass
    import concourse.tile as tile
    from concourse.bass import mybir

    dt = mybir.dt
    fp32, bf16, i32 = dt.float32, dt.bfloat16, dt.int32
    G = cap // 128
    if ncols is None:
        ncols = cap

    nc = bacc.Bacc("TRN2", target_bir_lowering=False, debug=False, num_devices=E)

    xT = nc.dram_tensor("xT", [HB, 128, cap], bf16, kind="ExternalInput").ap()
    w1b = nc.dram_tensor("w1b", [FB, 128, H], bf16, kind="ExternalInput").ap()
    v1b = nc.dram_tensor("v1b", [FB, 128, H], bf16, kind="ExternalInput").ap()
    w2b = nc.dram_tensor("w2b", [IC, FB, 128, ICW], bf16, kind="ExternalInput").ap()
    scale_sel = nc.dram_tensor("scale_sel", [128, G], fp32, kind="ExternalInput").ap()
    tokidx = nc.dram_tensor("tokidx", [128, G], i32, kind="ExternalInput").ap()
    out = nc.dram_tensor("out", [TSH, H], bf16, kind="ExternalOutput").ap()

    with tile.TileContext(nc) as tc:
        with (
            tc.tile_pool(name="xp", bufs=1) as xp,
            tc.tile_pool(name="w1p", bufs=wbufs) as w1p,
            tc.tile_pool(name="v1p", bufs=wbufs) as v1p,
            tc.tile_pool(name="w2p", bufs=2) as w2p,
            tc.tile_pool(name="hp", bufs=1) as hp,
            tc.tile_pool(name="sip", bufs=3) as sip,
            tc.tile_pool(name="yp", bufs=4) as yp,
            tc.tile_pool(name="zp", bufs=1) as zp,
            tc.tile_pool(name="scp", bufs=2) as scp,
            tc.tile_pool(name="ps_g", bufs=2, space=bass.MemorySpace.PSUM) as ps_g,
            tc.tile_pool(name="ps_u", bufs=2, space=bass.MemorySpace.PSUM) as ps_u,
            tc.tile_pool(name="dram", bufs=2, space="DRAM") as dram,
        ):
          # zero source tile, written once, reused by every rep's zero-fill
          if combine:
              zt = zp.tile([128, H], bf16)
              nc.vector.memset(zt[:], 0.0)

          for _rep in range(reps):
            if combine:
                y_dram = dram.tile([T + 128, H], bf16, name="y_dram",
                                   tag="ydram")
                # zero the scatter target (incl. trash rows) on the Act HWDGE
                # queue so it never blocks the SP queue's weight prefetches
                nrb = (T // 128) if ztrim else ((T + 128) // 128)
                for rb in range(nrb):
                    nc.scalar.dma_start(
                        y_dram[rb * 128:(rb + 1) * 128, :], zt[:])

            x_all = xp.tile([128, HB * cap], bf16)
            for hb in range(HB):
                nc.sync.dma_start(x_all[:, hb * cap:(hb + 1) * cap], xT[hb])
            sc = scp.tile([128, G], fp32, tag="tsc")
            ti = scp.tile([128, G], i32, tag="tti")
            nc.sync.dma_start(sc[:], scale_sel)
            nc.sync.dma_start(ti[:], tokidx)

            # ---- phase 1: h = silu(x @ w1.T) * (x @ v1.T), all selected tokens
            h_all = hp.tile([128, FB * cap], bf16)
            if not phase1:
                nc.vector.memset(h_all[:], 0.0)
            for fb in range(FB if phase1 else 0):
                w1_sb = w1p.tile([128, H], bf16)
                v1_sb = v1p.tile([128, H], bf16)
                nc.sync.dma_start(w1_sb[:], w1b[fb])
                nc.sync.dma_start(v1_sb[:], v1b[fb])
                chs = _chunks(ncols) if pair else None
                if not pair:
                    for (c0, cw) in _chunks(ncols):
                        gate = ps_g.tile([128, cw], mybir.dt.float32,
                                         name="gate0", tag="g0")
                        up = ps_u.tile([128, cw], mybir.dt.float32,
                                       name="up0", tag="u0")
                        for hb in range(HB):
                            lhs_w = w1_sb[:, hb * 128:(hb + 1) * 128]
                            lhs_v = v1_sb[:, hb * 128:(hb + 1) * 128]
                            rhs = x_all[:, hb * cap + c0: hb * cap + c0 + cw]
                            nc.tensor.matmul(gate[:], lhs_w, rhs,
                                             start=(hb == 0), stop=(hb == HB - 1))
                            nc.tensor.matmul(up[:], lhs_v, rhs,
                                             start=(hb == 0), stop=(hb == HB - 1))
                        silu = sip.tile([128, cw], mybir.dt.float32)
                        nc.scalar.activation(silu[:], gate[:],
                                             mybir.ActivationFunctionType.Silu)
                        nc.vector.tensor_mul(
                            h_all[:, fb * cap + c0: fb * cap + c0 + cw],
                            silu[:], up[:])
                    continue
                gates = [ps_g.tile([128, cw], mybir.dt.float32,
                                   name=f"gate{ci}", tag=f"g{ci}")
                         for ci, (c0, cw) in enumerate(chs)]
                ups = [ps_u.tile([128, cw], mybir.dt.float32,
                                 name=f"up{ci}", tag=f"u{ci}")
                       for ci, (c0, cw) in enumerate(chs)]
                for hb in range(HB):
                    lhs_w = w1_sb[:, hb * 128:(hb + 1) * 128]
                    lhs_v = v1_sb[:, hb * 128:(hb + 1) * 128]
                    # both column chunks consecutively under one stationary
                    # load, so walrus ldw-opt can drop the redundant reload
                    for ci, (c0, cw) in enumerate(chs):
                        rhs = x_all[:, hb * cap + c0: hb * cap + c0 + cw]
                        nc.tensor.matmul(gates[ci][:], lhs_w, rhs,
                                         start=(hb == 0), stop=(hb == HB - 1))
                    for ci, (c0, cw) in enumerate(chs):
                        rhs = x_all[:, hb * cap + c0: hb * cap + c0 + cw]
                        nc.tensor.matmul(ups[ci][:], lhs_v, rhs,
                                         start=(hb == 0), stop=(hb == HB - 1))
                for ci, (c0, cw) in enumerate(chs):
                    silu = sip.tile([128, cw], mybir.dt.float32)
                    nc.scalar.activation(silu[:], gates[ci][:],
                                         mybir.ActivationFunctionType.Silu)
                    nc.vector.tensor_mul(
                        h_all[:, fb * cap + c0: fb * cap + c0 + cw],
                        silu[:], ups[ci][:])

            # ---- phase 2: y = (h @ w2) * scale, scattered to token rows
            for ic in range(IC if phase2 else 0):
                w2_sb = w2p.tile([128, FB * ICW], bf16)
                for fb in range(FB):
                    nc.sync.dma_start(w2_sb[:, fb * ICW:(fb + 1) * ICW],
                                      w2b[ic, fb])
                for g in range(G):
                    ypsum = ps_g.tile([128, ICW], mybir.dt.float32,
                                      name="ypsum",
                                      tag="g0" if yshare else "yp")
                    for fb in range(FB):
                        lhs_h = h_all[:, fb * cap + g * 128:
                                      fb * cap + g * 128 + 128]
                        rhs_w = w2_sb[:, fb * ICW:(fb + 1) * ICW]
                        nc.tensor.matmul(ypsum[:], lhs_h, rhs_w,
                                         start=(fb == 0), stop=(fb == FB - 1))
                    y_sb = yp.tile([128, ICW], bf16)
                    nc.vector.tensor_scalar_mul(y_sb[:], ypsum[:],
                                                sc[:, g:g + 1])
                    if combine:
                        nc.gpsimd.indirect_dma_start(
                            out=y_dram[:],
                            out_offset=bass.IndirectOffsetOnAxis(
                                ap=ti[:, g:g + 1], axis=0),
                            in_=y_sb[:],
                            in_offset=None,
                            element_offset=ic * ICW,
                        )

            if combine and phase2:
                # one ReduceScatter over token rows; core r keeps its
                # 256-row shard. The final copy to the ExternalOutput is
                # issued from the gpsimd queue: its wait on the collective
                # must not block the SP queue's next-rep prefetches.
                y_sh = dram.tile([TSH, H], bf16, name="y_sh", tag="ysh")
                nc.gpsimd.collective_compute(
                    "ReduceScatter",
                    mybir.AluOpType.add,
                    replica_groups=[list(range(E))],
                    ins=[y_dram[:T, :]],
                    outs=[y_sh.opt()],
                )
                nc.gpsimd.dma_start(out, y_sh[:])
    nc.compile()
    if dedup:
        _dedup_ldweights(nc)
    return nc


def _dedup_ldweights(nc):
    # The PE array keeps its stationary operand across matmuls, so a
    # Ldweights identical to the previous one on the PE queue is a pure
    # reload of the same weights — drop it. Only instructions that carry no
    # semaphore waits/updates are removed. EventSemaphore/Drain/Matmult do
    # not alter the loaded weights and are transparent to the tracking.
    def sig_of(ap):
        mr = ap.memref
        mname = mr.name if hasattr(mr, "name") else str(mr)
        return (mname, ap.offset, str(ap.ap))

    fn = nc.m.functions[0]
    removed = 0
    for blk in fn.blocks:
        prev_sig = None
        keep = []
        for inst in blk.instructions:
            tn = type(inst).__name__
            if tn == "InstLdweights":
                s = sig_of(inst.ins[0])
                si = inst.sync_info
                clean = not si or (len(si.on_wait) == 0 and
                                   len(si.on_update) == 0)
                if s == prev_sig and clean:
                    removed += 1
                    continue
                prev_sig = s
            elif tn not in ("InstMatmult", "InstEventSemaphore", "InstDrain"):
                prev_sig = None
            keep.append(inst)
        blk.instructions[:] = keep
    return removed


def _prep_inputs(x, top_weights, top_experts, w1, v1, w2):
    import ml_dtypes

    bf16 = ml_dtypes.bfloat16
    x2 = np.asarray(x, np.float32).reshape(T, H)

    scale = np.zeros((T, E), np.float32)
    np.add.at(scale, (np.arange(T)[:, None], np.asarray(top_experts, np.int64)),
              np.asarray(top_weights, np.float32))

    toks = [np.nonzero(scale[:, c] != 0.0)[0] for c in range(E)]
    maxn = max(max(len(t) for t in toks), 1)
    cap = ((maxn + 127) // 128) * 128
    ncols = maxn
    G = cap // 128

    in_maps = []
    for c in range(E):
        tok = toks[c]
        n = len(tok)
        gat = np.zeros(cap, np.int64)
        gat[:n] = tok
        sct = np.full(cap, T, np.int32)
        sct[:n] = tok.astype(np.int32)
        scv = np.zeros(cap, np.float32)
        scv[:n] = scale[tok, c]

        xsel = x2[gat]                                  # [cap, H]
        xTs = np.ascontiguousarray(xsel.T).astype(bf16) # [H, cap]

        w1c = np.asarray(w1[c], np.float32)
        v1c = np.asarray(v1[c], np.float32)
        w2c = np.asarray(w2[c], np.float32)
        w1r = np.ascontiguousarray(
            w1c.reshape(FB, 128, HB, 128).transpose(0, 3, 2, 1)).astype(bf16)
        v1r = np.ascontiguousarray(
            v1c.reshape(FB, 128, HB, 128).transpose(0, 3, 2, 1)).astype(bf16)
        w2r = np.ascontiguousarray(
            w2c.reshape(FB, 128, IC, ICW).transpose(2, 0, 1, 3)).astype(bf16)
        in_maps.append({
            "xT": xTs.reshape(HB, 128, cap),
            "w1b": w1r.reshape(FB, 128, H),
            "v1b": v1r,
            "w2b": w2r,
            "scale_sel": np.ascontiguousarray(scv.reshape(G, 128).T),
            "tokidx": np.ascontiguousarray(sct.reshape(G, 128).T),
        })
    return cap, ncols, in_maps


def _assemble(results):
    full = np.concatenate(
        [np.asarray(results[c]["out"], np.float32) for c in range(E)], axis=0)
    return full.reshape(B, S, H)


def kernel(x, weights, top_weights, top_experts, w1, v1, w2):
    import sys
    if "/opt/trn_rl_repo" not in sys.path:
        sys.path.insert(0, "/opt/trn_rl_repo")
    from concourse.bass_utils import run_bass_kernel_spmd

    cap, ncols, in_maps = _prep_inputs(x, top_weights, top_experts, w1, v1, w2)
    key = ("nc", cap, ncols)
    if key not in _STATE:
        _STATE[key] = _build_nc(cap, ncols=ncols)
        _STATE["nc"] = _STATE[key]
        _STATE["cap"] = cap
    nc = _STATE[key]

    res = run_bass_kernel_spmd(nc, in_maps, core_ids=list(range(E)))
    return _assemble(res.results)



# revision 7
# speedup vs baseline: 1.0592x; 1.0592x over previous
import numpy as np

E, F, H = 8, 4096, 2048
B, S, K = 2, 1024, 4
T = B * S

FB = F // 128       # 32 f-tiles
HB = H // 128       # 16 h-tiles
ICW = 256           # output column chunk
IC = H // ICW       # 8 output col chunks
TSH = T // E        # 256 rows per core after ReduceScatter

_STATE = {}


def _chunks(cap):
    out, c0 = [], 0
    while c0 < cap:
        cw = min(512, cap - c0)
        out.append((c0, cw))
        c0 += cw
    return out


def _build_nc(cap, reps=1, ncols=None, combine=True, phase1=True, phase2=True,
              pair=True, dedup=True, yshare=True, wbufs=2, ztrim=True,
              w2pre=3):
    # ncols: real (unpadded) token columns; cols [ncols, cap) of h are never
    # computed — their phase-2 outputs land in the trash row via the scatter
    # index padding, so garbage there is harmless.
    #
    # Combine path: each rep scatters its expert's token rows into a
    # double-buffered [T+128, H] DRAM staging buffer, then one ReduceScatter
    # sums across the 8 cores and writes each core's 256-row shard directly
    # into the ExternalOutput. Double buffering (dram pool bufs=2) lets rep
    # i's ReduceScatter run concurrently with rep i+1's compute, so in
    # steady state the collective is fully hidden.
    import concourse.bacc as bacc
    import concourse.bass as bass
    import concourse.tile as tile
    from concourse.bass import mybir

    dt = mybir.dt
    fp32, bf16, i32 = dt.float32, dt.bfloat16, dt.int32
    G = cap // 128
    if ncols is None:
        ncols = cap

    nc = bacc.Bacc("TRN2", target_bir_lowering=False, debug=False, num_devices=E)

    xT = nc.dram_tensor("xT", [HB, 128, cap], bf16, kind="ExternalInput").ap()
    w1b = nc.dram_tensor("w1b", [FB, 128, H], bf16, kind="ExternalInput").ap()
    v1b = nc.dram_tensor("v1b", [FB, 128, H], bf16, kind="ExternalInput").ap()
    w2b = nc.dram_tensor("w2b", [IC, FB, 128, ICW], bf16, kind="ExternalInput").ap()
    scale_sel = nc.dram_tensor("scale_sel", [128, G], fp32, kind="ExternalInput").ap()
    tokidx = nc.dram_tensor("tokidx", [128, G], i32, kind="ExternalInput").ap()
    out = nc.dram_tensor("out", [TSH, H], bf16, kind="ExternalOutput").ap()

    with tile.TileContext(nc) as tc:
        with (
            tc.tile_pool(name="xp", bufs=1) as xp,
            tc.tile_pool(name="w1p", bufs=wbufs) as w1p,
            tc.tile_pool(name="v1p", bufs=wbufs) as v1p,
            tc.tile_pool(name="w2p", bufs=3) as w2p,
            tc.tile_pool(name="hp", bufs=1) as hp,
            tc.tile_pool(name="sip", bufs=3) as sip,
            tc.tile_pool(name="yp", bufs=4) as yp,
            tc.tile_pool(name="zp", bufs=1) as zp,
            tc.tile_pool(name="scp", bufs=2) as scp,
            tc.tile_pool(name="ps_g", bufs=2, space=bass.MemorySpace.PSUM) as ps_g,
            tc.tile_pool(name="ps_u", bufs=2, space=bass.MemorySpace.PSUM) as ps_u,
            tc.tile_pool(name="dram", bufs=2, space="DRAM") as dram,
        ):
          # zero source tile, written once, reused by every rep's zero-fill
          if combine:
              zt = zp.tile([128, H], bf16)
              nc.vector.memset(zt[:], 0.0)

          for _rep in range(reps):
            if combine:
                y_dram = dram.tile([T + 128, H], bf16, name="y_dram",
                                   tag="ydram")
                # zero the scatter target (incl. trash rows) on the Act HWDGE
                # queue so it never blocks the SP queue's weight prefetches
                nrb = (T // 128) if ztrim else ((T + 128) // 128)
                for rb in range(nrb):
                    nc.scalar.dma_start(
                        y_dram[rb * 128:(rb + 1) * 128, :], zt[:])

            x_all = xp.tile([128, HB * cap], bf16)
            for hb in range(HB):
                nc.sync.dma_start(x_all[:, hb * cap:(hb + 1) * cap], xT[hb])
            sc = scp.tile([128, G], fp32, tag="tsc")
            ti = scp.tile([128, G], i32, tag="tti")
            nc.sync.dma_start(sc[:], scale_sel)
            nc.sync.dma_start(ti[:], tokidx)

            # Prefetch the first w2 column-chunks at rep start on the Act
            # HWDGE queue. Their buffer-release waits resolve during the
            # PREVIOUS rep's phase 2, so the transfers overlap compute and
            # phase 2 never stalls on its first weight load. Later chunks
            # are issued inside the ic loop (after this rep's silus in the
            # Act stream), so their waits never block phase-1 activations.
            w2_tiles = {}
            if phase2:
                for ic in range(min(w2pre, IC)):
                    t = w2p.tile([128, FB * ICW], bf16, name=f"w2sb{ic}",
                                 tag="w2")
                    for fb in range(FB):
                        nc.scalar.dma_start(t[:, fb * ICW:(fb + 1) * ICW],
                                            w2b[ic, fb])
                    w2_tiles[ic] = t

            # ---- phase 1: h = silu(x @ w1.T) * (x @ v1.T), all selected tokens
            h_all = hp.tile([128, FB * cap], bf16)
            if not phase1:
                nc.vector.memset(h_all[:], 0.0)
            for fb in range(FB if phase1 else 0):
                w1_sb = w1p.tile([128, H], bf16)
                v1_sb = v1p.tile([128, H], bf16)
                nc.sync.dma_start(w1_sb[:], w1b[fb])
                nc.sync.dma_start(v1_sb[:], v1b[fb])
                chs = _chunks(ncols) if pair else None
                if not pair:
                    for (c0, cw) in _chunks(ncols):
                        gate = ps_g.tile([128, cw], mybir.dt.float32,
                                         name="gate0", tag="g0")
                        up = ps_u.tile([128, cw], mybir.dt.float32,
                                       name="up0", tag="u0")
                        for hb in range(HB):
                            lhs_w = w1_sb[:, hb * 128:(hb + 1) * 128]
                            lhs_v = v1_sb[:, hb * 128:(hb + 1) * 128]
                            rhs = x_all[:, hb * cap + c0: hb * cap + c0 + cw]
                            nc.tensor.matmul(gate[:], lhs_w, rhs,
                                             start=(hb == 0), stop=(hb == HB - 1))
                            nc.tensor.matmul(up[:], lhs_v, rhs,
                                             start=(hb == 0), stop=(hb == HB - 1))
                        silu = sip.tile([128, cw], mybir.dt.float32)
                        nc.scalar.activation(silu[:], gate[:],
                                             mybir.ActivationFunctionType.Silu)
                        nc.vector.tensor_mul(
                            h_all[:, fb * cap + c0: fb * cap + c0 + cw],
                            silu[:], up[:])
                    continue
                gates = [ps_g.tile([128, cw], mybir.dt.float32,
                                   name=f"gate{ci}", tag=f"g{ci}")
                         for ci, (c0, cw) in enumerate(chs)]
                ups = [ps_u.tile([128, cw], mybir.dt.float32,
                                 name=f"up{ci}", tag=f"u{ci}")
                       for ci, (c0, cw) in enumerate(chs)]
                for hb in range(HB):
                    lhs_w = w1_sb[:, hb * 128:(hb + 1) * 128]
                    lhs_v = v1_sb[:, hb * 128:(hb + 1) * 128]
                    # both column chunks consecutively under one stationary
                    # load, so walrus ldw-opt can drop the redundant reload
                    for ci, (c0, cw) in enumerate(chs):
                        rhs = x_all[:, hb * cap + c0: hb * cap + c0 + cw]
                        nc.tensor.matmul(gates[ci][:], lhs_w, rhs,
                                         start=(hb == 0), stop=(hb == HB - 1))
                    for ci, (c0, cw) in enumerate(chs):
                        rhs = x_all[:, hb * cap + c0: hb * cap + c0 + cw]
                        nc.tensor.matmul(ups[ci][:], lhs_v, rhs,
                                         start=(hb == 0), stop=(hb == HB - 1))
                for ci, (c0, cw) in enumerate(chs):
                    silu = sip.tile([128, cw], mybir.dt.float32)
                    nc.scalar.activation(silu[:], gates[ci][:],
                                         mybir.ActivationFunctionType.Silu)
                    nc.vector.tensor_mul(
                        h_all[:, fb * cap + c0: fb * cap + c0 + cw],
                        silu[:], ups[ci][:])

            # ---- phase 2: y = (h @ w2) * scale, scattered to token rows
            for ic in range(IC if phase2 else 0):
                if ic in w2_tiles:
                    w2_sb = w2_tiles.pop(ic)
                else:
                    w2_sb = w2p.tile([128, FB * ICW], bf16, name=f"w2sb{ic}",
                                     tag="w2")
                    for fb in range(FB):
                        nc.scalar.dma_start(
                            w2_sb[:, fb * ICW:(fb + 1) * ICW], w2b[ic, fb])
                for g in range(G):
                    ypsum = ps_g.tile([128, ICW], mybir.dt.float32,
                                      name="ypsum",
                                      tag="g0" if yshare else "yp")
                    for fb in range(FB):
                        lhs_h = h_all[:, fb * cap + g * 128:
                                      fb * cap + g * 128 + 128]
                        rhs_w = w2_sb[:, fb * ICW:(fb + 1) * ICW]
                        nc.tensor.matmul(ypsum[:], lhs_h, rhs_w,
                                         start=(fb == 0), stop=(fb == FB - 1))
                    y_sb = yp.tile([128, ICW], bf16)
                    nc.vector.tensor_scalar_mul(y_sb[:], ypsum[:],
                                                sc[:, g:g + 1])
                    if combine:
                        nc.gpsimd.indirect_dma_start(
                            out=y_dram[:],
                            out_offset=bass.IndirectOffsetOnAxis(
                                ap=ti[:, g:g + 1], axis=0),
                            in_=y_sb[:],
                            in_offset=None,
                            element_offset=ic * ICW,
                        )

            if combine and phase2:
                # one ReduceScatter over token rows; core r keeps its
                # 256-row shard. The final copy to the ExternalOutput is
                # issued from the gpsimd queue: its wait on the collective
                # must not block the SP queue's next-rep prefetches.
                y_sh = dram.tile([TSH, H], bf16, name="y_sh", tag="ysh")
                nc.gpsimd.collective_compute(
                    "ReduceScatter",
                    mybir.AluOpType.add,
                    replica_groups=[list(range(E))],
                    ins=[y_dram[:T, :]],
                    outs=[y_sh.opt()],
                )
                nc.gpsimd.dma_start(out, y_sh[:])
    nc.compile()
    if dedup:
        _dedup_ldweights(nc)
    return nc


def _dedup_ldweights(nc):
    # The PE array keeps its stationary operand across matmuls, so a
    # Ldweights identical to the previous one on the PE queue is a pure
    # reload of the same weights — drop it. Only instructions that carry no
    # semaphore waits/updates are removed. EventSemaphore/Drain/Matmult do
    # not alter the loaded weights and are transparent to the tracking.
    def sig_of(ap):
        mr = ap.memref
        mname = mr.name if hasattr(mr, "name") else str(mr)
        return (mname, ap.offset, str(ap.ap))

    fn = nc.m.functions[0]
    removed = 0
    for blk in fn.blocks:
        prev_sig = None
        keep = []
        for inst in blk.instructions:
            tn = type(inst).__name__
            if tn == "InstLdweights":
                s = sig_of(inst.ins[0])
                si = inst.sync_info
                clean = not si or (len(si.on_wait) == 0 and
                                   len(si.on_update) == 0)
                if s == prev_sig and clean:
                    removed += 1
                    continue
                prev_sig = s
            elif tn not in ("InstMatmult", "InstEventSemaphore", "InstDrain"):
                prev_sig = None
            keep.append(inst)
        blk.instructions[:] = keep
    return removed


def _prep_inputs(x, top_weights, top_experts, w1, v1, w2):
    import ml_dtypes

    bf16 = ml_dtypes.bfloat16
    x2 = np.asarray(x, np.float32).reshape(T, H)

    scale = np.zeros((T, E), np.float32)
    np.add.at(scale, (np.arange(T)[:, None], np.asarray(top_experts, np.int64)),
              np.asarray(top_weights, np.float32))

    toks = [np.nonzero(scale[:, c] != 0.0)[0] for c in range(E)]
    maxn = max(max(len(t) for t in toks), 1)
    cap = ((maxn + 127) // 128) * 128
    ncols = maxn
    G = cap // 128

    in_maps = []
    for c in range(E):
        tok = toks[c]
        n = len(tok)
        gat = np.zeros(cap, np.int64)
        gat[:n] = tok
        sct = np.full(cap, T, np.int32)
        sct[:n] = tok.astype(np.int32)
        scv = np.zeros(cap, np.float32)
        scv[:n] = scale[tok, c]

        xsel = x2[gat]                                  # [cap, H]
        xTs = np.ascontiguousarray(xsel.T).astype(bf16) # [H, cap]

        w1c = np.asarray(w1[c], np.float32)
        v1c = np.asarray(v1[c], np.float32)
        w2c = np.asarray(w2[c], np.float32)
        w1r = np.ascontiguousarray(
            w1c.reshape(FB, 128, HB, 128).transpose(0, 3, 2, 1)).astype(bf16)
        v1r = np.ascontiguousarray(
            v1c.reshape(FB, 128, HB, 128).transpose(0, 3, 2, 1)).astype(bf16)
        w2r = np.ascontiguousarray(
            w2c.reshape(FB, 128, IC, ICW).transpose(2, 0, 1, 3)).astype(bf16)
        in_maps.append({
            "xT": xTs.reshape(HB, 128, cap),
            "w1b": w1r.reshape(FB, 128, H),
            "v1b": v1r,
            "w2b": w2r,
            "scale_sel": np.ascontiguousarray(scv.reshape(G, 128).T),
            "tokidx": np.ascontiguousarray(sct.reshape(G, 128).T),
        })
    return cap, ncols, in_maps


def _assemble(results):
    full = np.concatenate(
        [np.asarray(results[c]["out"], np.float32) for c in range(E)], axis=0)
    return full.reshape(B, S, H)


def kernel(x, weights, top_weights, top_experts, w1, v1, w2):
    import sys
    if "/opt/trn_rl_repo" not in sys.path:
        sys.path.insert(0, "/opt/trn_rl_repo")
    from concourse.bass_utils import run_bass_kernel_spmd

    cap, ncols, in_maps = _prep_inputs(x, top_weights, top_experts, w1, v1, w2)
    key = ("nc", cap, ncols)
    if key not in _STATE:
        _STATE[key] = _build_nc(cap, ncols=ncols)
        _STATE["nc"] = _STATE[key]
        _STATE["cap"] = cap
    nc = _STATE[key]

    res = run_bass_kernel_spmd(nc, in_maps, core_ids=list(range(E)))
    return _assemble(res.results)

